# revision 9
# baseline (speedup 1.0000x reference)
"""Bottleneck-transformer block on 8 TRN2 NeuronCores.

Sharding: data-parallel over batch (B=64 -> 8 elements/core), weights
replicated; no collectives. BatchNorms are folded into conv weights on
the host. Device kernel per batch element:
  conv1+bn1+relu -> q/k (natural layout), v computed transposed ->
  attention with transposed logits (softmax along the partition axis:
  exp on ACT, column sums via ones-matmul on PE, 1/sum broadcast via a
  K=1 outer-product matmul) -> bn2+relu (folded into v path + bias) ->
  conv3+shortcut fused into one PSUM accumulation group + final relu.
All matmuls run as float32r (full-rate fp32 mode for free dim >= 256).
"""

import numpy as np

import concourse.bass as bass
import concourse.mybir as mybir
from concourse import bacc
from concourse.tile import TileContext
from concourse.bass_utils import run_bass_kernel_spmd

EPS = 1e-5
NCORES = 8
BLOC = 8          # batch elements per core
NT = 256          # tokens per element (16*16)
F32 = mybir.dt.float32
F32R = mybir.dt.float32r

_STATE = {}


def _r(w):
    """[K, M] weight -> [128, K//128, M] (partition-major lhsT layout)."""
    k, m = w.shape
    return np.ascontiguousarray(
        w.reshape(k // 128, 128, m).transpose(1, 0, 2)
    ).astype(np.float32)


def _b(v):
    """[C] bias -> [128, C//128] (partition-major per-m-tile scalars)."""
    return np.ascontiguousarray(v.reshape(-1, 128).T).astype(np.float32)


def _build_nc(reps=1):
    nc = bacc.Bacc("TRN2", target_bir_lowering=False, debug=False,
                   num_devices=NCORES)

    x_d = nc.declare_dram_parameter("x", [BLOC, 128, 8, NT], F32R, isOutput=False)
    w1_d = nc.declare_dram_parameter("w1t", [128, 8, 512], F32R, isOutput=False)
    qw_d = nc.declare_dram_parameter("qwt", [128, 4, 512], F32R, isOutput=False)
    kw_d = nc.declare_dram_parameter("kwt", [128, 4, 512], F32R, isOutput=False)
    vw_d = nc.declare_dram_parameter("vwt", [128, 4, 512], F32R, isOutput=False)
    w3_d = nc.declare_dram_parameter("w3t", [128, 4, 2048], F32R, isOutput=False)
    ws_d = nc.declare_dram_parameter("wsct", [128, 8, 2048], F32R, isOutput=False)
    pos_d = nc.declare_dram_parameter("post", [128, 4, NT], F32R, isOutput=False)
    b1_d = nc.declare_dram_parameter("b1", [128, 4], F32, isOutput=False)
    qb_d = nc.declare_dram_parameter("qb", [128, 4], F32, isOutput=False)
    kb_d = nc.declare_dram_parameter("kb", [128, 4], F32, isOutput=False)
    bv_d = nc.declare_dram_parameter("bv2", [128, 4], F32, isOutput=False)
    bf_d = nc.declare_dram_parameter("bfin", [128, 16], F32, isOutput=False)
    out_d = nc.declare_dram_parameter("out", [BLOC, 16, 128, NT], F32,
                                      isOutput=True)

    RELU = mybir.ActivationFunctionType.Relu
    EXPF = mybir.ActivationFunctionType.Exp

    def mm(ps, lhsT, rhs, start, stop):
        nc.tensor.matmul(ps, lhsT, rhs, start=start, stop=stop)

    with TileContext(nc) as tc:
        with (
            tc.tile_pool(name="wp", bufs=1) as wp,
            tc.tile_pool(name="act", bufs=2) as act,
            tc.tile_pool(name="att", bufs=3) as att,
            tc.tile_pool(name="outp", bufs=4) as outp,
            tc.tile_pool(name="psA", bufs=6, space="PSUM") as psA,
            tc.tile_pool(name="psC", bufs=2, space="PSUM") as psC,
        ):
            W1 = wp.tile([128, 8, 512], F32R)
            QW = wp.tile([128, 4, 512], F32R)
            KW = wp.tile([128, 4, 512], F32R)
            VW = wp.tile([128, 4, 512], F32R)
            W3 = wp.tile([128, 4, 2048], F32R)
            WS = wp.tile([128, 8, 2048], F32R)
            POS = wp.tile([128, 4, NT], F32R)
            B1 = wp.tile([128, 4], F32)
            QB = wp.tile([128, 4], F32)
            KB = wp.tile([128, 4], F32)
            BV = wp.tile([128, 4], F32)
            BF = wp.tile([128, 16], F32)
            ONK = wp.tile([128, 1], F32R)
            ONM = wp.tile([1, 128], F32R)

            ONKF = wp.tile([128, 1], F32)
            ONMF = wp.tile([1, 128], F32)
            nc.vector.memset(ONKF, 1.0)
            nc.vector.memset(ONMF, 1.0)
            nc.vector.tensor_copy(out=ONK, in_=ONKF)
            nc.vector.tensor_copy(out=ONM, in_=ONMF)

            for e in range(BLOC * reps):
                rep, e = divmod(e, BLOC)
                xe = act.tile([128, 8, NT], F32R, tag="xe")
                if e == 0 and rep == 0:
                    nc.sync.dma_start(out=B1, in_=b1_d[:])
                    for k in range(8):
                        nc.sync.dma_start(out=W1[:, k, :], in_=w1_d[:, k, :])
                        nc.sync.dma_start(out=xe[:, k, :], in_=x_d[e, :, k, :])
                    nc.sync.dma_start(out=QW, in_=qw_d[:])
                    nc.sync.dma_start(out=QB, in_=qb_d[:])
                    nc.sync.dma_start(out=KW, in_=kw_d[:])
                    nc.sync.dma_start(out=KB, in_=kb_d[:])
                    nc.sync.dma_start(out=VW, in_=vw_d[:])
                    nc.sync.dma_start(out=POS, in_=pos_d[:])
                    nc.sync.dma_start(out=BV, in_=bv_d[:])
                    nc.sync.dma_start(out=BF, in_=bf_d[:])
                    for m in range(4):
                        nc.sync.dma_start(out=W3[:, :, m * 512:(m + 1) * 512],
                                          in_=w3_d[:, :, m * 512:(m + 1) * 512])
                        nc.sync.dma_start(out=WS[:, :, m * 512:(m + 1) * 512],
                                          in_=ws_d[:, :, m * 512:(m + 1) * 512])
                else:
                    nc.sync.dma_start(out=xe, in_=x_d[e])

                # conv1 + bn1 + relu -> out1 [c(4x128), n]
                out1 = act.tile([128, 4, NT], F32R, tag="out1")
                for m in range(4):
                    ps = psA.tile([128, NT], F32, tag="mm")
                    for k in range(8):
                        mm(ps, W1[:, k, m * 128:(m + 1) * 128], xe[:, k, :],
                           k == 0, k == 7)
                    nc.scalar.activation(out1[:, m, :], ps, RELU,
                                         bias=B1[:, m:m + 1])

                # q, k projections (natural layout) with bias
                qt = act.tile([128, 4, NT], F32R, tag="qt", bufs=1)
                kt = act.tile([128, 4, NT], F32R, tag="kt", bufs=1)
                for m in range(4):
                    ps = psA.tile([128, NT], F32, tag="mm")
                    for k in range(4):
                        mm(ps, QW[:, k, m * 128:(m + 1) * 128], out1[:, k, :],
                           k == 0, k == 3)
                    nc.vector.tensor_scalar_add(qt[:, m, :], ps, QB[:, m:m + 1])
                    ps2 = psA.tile([128, NT], F32, tag="mm")
                    for k in range(4):
                        mm(ps2, KW[:, k, m * 128:(m + 1) * 128], out1[:, k, :],
                           k == 0, k == 3)
                    nc.vector.tensor_scalar_add(kt[:, m, :], ps2, KB[:, m:m + 1])

                # v, transposed: vT [tok(2x128), c(512)]
                vtt = act.tile([128, 2, 512], F32R, tag="vtt", bufs=1)
                for mt in range(2):
                    ps = psA.tile([128, 512], F32, tag="mm")
                    for k in range(4):
                        mm(ps, out1[:, k, mt * 128:(mt + 1) * 128], VW[:, k, :],
                           k == 0, k == 3)
                    nc.vector.tensor_copy(out=vtt[:, mt, :], in_=ps)

                # attention, stage-grouped across heads; logits transposed
                out2 = act.tile([128, 4, NT], F32R, tag="out2")
                exts, recs, rcbs, psos = [], [], [], []
                for h in range(4):
                    ext = att.tile([128, 2, NT], F32R, tag="ext", bufs=4,
                                   name=f"ext{h}")
                    for mt in range(2):
                        psl = psA.tile([128, NT], F32, tag="mm")
                        mm(psl, kt[:, h, mt * 128:(mt + 1) * 128], qt[:, h, :],
                           True, False)
                        mm(psl, qt[:, h, mt * 128:(mt + 1) * 128], POS[:, h, :],
                           False, True)
                        nc.scalar.activation(ext[:, mt, :], psl, EXPF)
                    exts.append(ext)
                for h in range(4):
                    pss = psC.tile([1, NT], F32, tag="sum")
                    mm(pss, ONK, exts[h][:, 0, :], True, False)
                    mm(pss, ONK, exts[h][:, 1, :], False, True)
                    rec = att.tile([1, NT], F32R, tag="rec", bufs=4,
                                   name=f"rec{h}")
                    with nc.allow_low_precision(reason="softmax 1/sum in tf32"):
                        nc.vector.reciprocal(out=rec, in_=pss)
                    recs.append(rec)
                for h in range(4):
                    psr = psA.tile([128, NT], F32, tag="mm")
                    mm(psr, ONM, recs[h], True, True)
                    rcb = att.tile([128, NT], F32, tag="rcb", bufs=4,
                                   name=f"rcb{h}")
                    nc.scalar.copy(out=rcb, in_=psr)
                    rcbs.append(rcb)
                for h in range(4):
                    pso = psA.tile([128, NT], F32, tag="mm")
                    for mt in range(2):
                        mm(pso, vtt[:, mt, h * 128:(h + 1) * 128],
                           exts[h][:, mt, :], mt == 0, mt == 1)
                    tmp = att.tile([128, NT], F32, tag="tmp", bufs=2,
                                   name=f"tmp{h}")
                    nc.vector.tensor_mul(out=tmp, in0=pso, in1=rcbs[h])
                    nc.scalar.activation(out2[:, h, :], tmp, RELU,
                                         bias=BV[:, h:h + 1])

                # conv3 + shortcut fused, + bn3/scbn biases + relu
                for m in range(16):
                    ps = psA.tile([128, NT], F32, tag="mm")
                    for k in range(4):
                        mm(ps, W3[:, k, m * 128:(m + 1) * 128], out2[:, k, :],
                           k == 0, False)
                    for k in range(8):
                        mm(ps, WS[:, k, m * 128:(m + 1) * 128], xe[:, k, :],
                           False, k == 7)
                    ot = outp.tile([128, NT], F32, tag="ot")
                    nc.scalar.activation(ot, ps, RELU, bias=BF[:, m:m + 1])
                    nc.sync.dma_start(out=out_d[e, m], in_=ot)

    nc.compile()
    return nc


def _prep_shared(i):
    s1 = (i["bn1_g"] / np.sqrt(i["bn1_v"] + EPS)).astype(np.float64)
    w1 = i["conv1_w"].astype(np.float64) * s1[:, None]
    b1 = i["bn1_b"].astype(np.float64) - i["bn1_m"].astype(np.float64) * s1

    s2 = (i["bn2_g"] / np.sqrt(i["bn2_v"] + EPS)).astype(np.float64)
    b2 = i["bn2_b"].astype(np.float64) - i["bn2_m"].astype(np.float64) * s2
    vw = i["v_w"].astype(np.float64) * s2[:, None]
    bv2 = i["v_b"].astype(np.float64) * s2 + b2

    s3 = (i["bn3_g"] / np.sqrt(i["bn3_v"] + EPS)).astype(np.float64)
    w3 = i["conv3_w"].astype(np.float64) * s3[:, None]
    b3 = i["bn3_b"].astype(np.float64) - i["bn3_m"].astype(np.float64) * s3

    ss = (i["scbn_g"] / np.sqrt(i["scbn_v"] + EPS)).astype(np.float64)
    wsc = i["sc_w"].astype(np.float64) * ss[:, None]
    bsc = (ss * (i["sc_b"].astype(np.float64) - i["scbn_m"].astype(np.float64))
           + i["scbn_b"].astype(np.float64))

    pos = (i["rel_h"] + i["rel_w"]).reshape(4, 128, NT)

    return {
        "w1t": _r(w1.T), "qwt": _r(i["q_w"].T), "kwt": _r(i["k_w"].T),
        "vwt": _r(vw.T), "w3t": _r(w3.T), "wsct": _r(wsc.T),
        "post": np.ascontiguousarray(pos.transpose(1, 0, 2)).astype(np.float32),
        "b1": _b(b1), "qb": _b(i["q_b"]), "kb": _b(i["k_b"]),
        "bv2": _b(bv2), "bfin": _b(b3 + bsc),
    }


def kernel(**inputs):
    if "nc" not in _STATE:
        _STATE["nc"] = _build_nc()
    nc = _STATE["nc"]

    shared = _prep_shared({k: np.asarray(v) for k, v in inputs.items()})
    x = np.asarray(inputs["x"], np.float32).reshape(64, 8, 128, NT)
    x = np.ascontiguousarray(x.transpose(0, 2, 1, 3))  # [B, 128, 8, NT]

    in_maps = []
    for c in range(NCORES):
        m = dict(shared)
        m["x"] = np.ascontiguousarray(x[c * BLOC:(c + 1) * BLOC])
        in_maps.append(m)

    res = run_bass_kernel_spmd(nc, in_maps, list(range(NCORES)))
    out = np.concatenate(
        [res.results[c]["out"].reshape(BLOC, 2048, 16, 16)
         for c in range(NCORES)], axis=0)
    return out.astype(np.float32)
